# revision 41
# baseline (speedup 1.0000x reference)
"""Trainium2 Bass kernel: full-sequence multi-head attention
(S=2048, DIM=1024, H=16, D=64) sharded across 8 NeuronCores with
tensor parallelism on heads (2 heads per core), zero device collectives.

v3 — ACT(exp)-bottleneck-centric schedule. Per-core program:

  phase 1 (8 rope passes of 512 cols): qkvT matmuls (PE, K=128) ->
    DVE evac+bias (bf16) -> rotate-half swap copies (ACT for the first
    four passes, which finish before any exp; DVE for the rest) ->
    DVE mul/mul/add with cos / signed-sin tables -> q_rot/k_rot.
    x arrives via 3 bundled wide DMAs per column-half (one per queue)
    into a single wide tile, so transfers start early and saturate HBM.
    v is computed as [seq, d] tiles with the ones-column trick (vAB).
  scores: per k-tile, two concurrent 64-row matmuls (head A rows 0:63,
    head B rows 64:127) -> [128,1024] PSUM -> ACT exp -> bf16 pt tile.
    ACT does nothing else between the first and last exp.
  attn@v: K=128 single-accumulation chains, one per (pair, cc, head):
    16 matmuls N=512 into one PSUM bank. Pair-0 cc0 rides the pair-0
    exp stream; pair-1's four chains ride the pair-1 exp stream
    together (4 PSUM banks) so almost nothing is left after the last
    exp. Head A evacs to outA (+den row), head B directly into outN
    rows 64:128 (+den row to outB) to satisfy the tensor_tensor
    matched-base-partition rule.
  normalize: two K=1 broadcast matmuls (mask row x denom row) ->
    [128,512] PSUM -> DVE fast reciprocal -> two DVE muls -> outN.
  proj: per 128-seq chunk, 2 matmuls vs wpT -> [128,1024] PSUM ->
    evac (DVE; ACT for the post-exp tail) -> DMA out bf16 partials.
  PE is pre-warmed with dummy matmuls so HAM reaches 8/8 before the
  first real matmul.

Host: y = sum_c y_c + b_proj (float64 accumulate). Host-side prep:
x pre-transposed, per-core head-sliced weights pre-transposed/bundled,
1/sqrt(D) folded into W_q/b_q, RoPE tables expanded to [128, S] with
the rotate-half sign folded into the sin table.
"""

import sys

if "/opt/trn_rl_repo" not in sys.path:
    sys.path.insert(0, "/opt/trn_rl_repo")

import numpy as np
import ml_dtypes

from concourse import bass, bacc, tile, bass_utils

mybir = bass.mybir
F32 = mybir.dt.float32
BF16 = mybir.dt.bfloat16
EXP = mybir.ActivationFunctionType.Exp
ADD = mybir.AluOpType.add
MULT = mybir.AluOpType.mult

S, DIM, H, D = 2048, 1024, 16, 64
N_CORES = 8
HPC = 2  # heads per core
DL = HPC * D  # local head dims = 128
NKT = S // 128  # 16 k tiles
NDT = DIM // 128  # 8 contraction tiles for qkv


def build():
    nc = bacc.Bacc("TRN2", target_bir_lowering=False, debug=False,
                   num_devices=N_CORES)

    xT_e = nc.dram_tensor("xT", [DIM, S], BF16, kind="ExternalInput").ap()
    # weight bundles: col block i holds dim-rows i*128:(i+1)*128 of W*T
    wqB_e = nc.dram_tensor("wqB", [128, DIM], BF16, kind="ExternalInput").ap()
    wkB_e = nc.dram_tensor("wkB", [128, DIM], BF16, kind="ExternalInput").ap()
    wvB_e = nc.dram_tensor("wvB", [128, DIM], BF16, kind="ExternalInput").ap()
    cosT_e = nc.dram_tensor("cosT", [DL, S], BF16, kind="ExternalInput").ap()
    sinTs_e = nc.dram_tensor("sinTs", [DL, S], BF16, kind="ExternalInput").ap()
    wpT_e = nc.dram_tensor("wpT", [DL, DIM], BF16, kind="ExternalInput").ap()
    bq_e = nc.dram_tensor("bq", [DL, 1], F32, kind="ExternalInput").ap()
    bk_e = nc.dram_tensor("bk", [DL, 1], F32, kind="ExternalInput").ap()
    bvb_e = nc.dram_tensor("bvb", [DL, DL], F32, kind="ExternalInput").ap()
    out_e = nc.dram_tensor("out", [S, DIM], BF16, kind="ExternalOutput").ap()

    with tile.TileContext(nc) as tc:
        with tc.tile_pool(name="persist", bufs=1) as pp, \
             tc.tile_pool(name="ps_sc", bufs=3, space="PSUM") as ps_sc, \
             tc.tile_pool(name="ps_sm", bufs=2, space="PSUM") as ps_sm, \
             tc.tile_pool(name="rope_t", bufs=6) as rtp, \
             tc.tile_pool(name="norm_t", bufs=4) as ntp, \
             tc.tile_pool(name="ysb", bufs=4) as ysbp:
            q_rot = pp.tile([128, S], BF16, tag="q_rot", name="q_rot")
            k_rot = pp.tile([128, S], BF16, tag="k_rot", name="k_rot")
            # per k-tile block of 130 cols: [vA(64) | 1 | vB(64) | 1]
            vAB = pp.tile([128, NKT * 130], BF16, tag="vAB", name="vAB")
            outA = pp.tile([65, S], BF16, tag="outA", name="outA")
            outB = pp.tile([65, S], BF16, tag="outB", name="outB")
            outN = pp.tile([128, S], BF16, tag="outN", name="outN")
            wpT = pp.tile([DL, DIM], BF16, tag="wpT", name="wpT")
            bq = pp.tile([DL, 1], F32, tag="bq", name="bq")
            bk = pp.tile([DL, 1], F32, tag="bk", name="bk")
            bvb = pp.tile([DL, DL], F32, tag="bvb", name="bvb")
            ones16 = pp.tile([128, 16], F32, tag="ones16", name="ones16")
            # broadcast masks live on partition 64 (same base partition as
            # the denominator rows in outA/outB): cols 0:128 = head-A mask,
            # cols 128:256 = head-B mask
            maskAB = pp.tile([65, 256], BF16, tag="maskAB", name="maskAB")
            warm = pp.tile([128, 512], BF16, tag="warm", name="warm")
            # pt pool opened BEFORE the phase-1 input pool so p1 can be
            # released mid-kernel (pools release in LIFO order)
            ptp_cm = tc.tile_pool(name="pt", bufs=44)
            ptp = ptp_cm.__enter__()
            p1_cm = tc.tile_pool(name="p1in", bufs=1)
            p1 = p1_cm.__enter__()
            # all 8 qkv contraction tiles in one wide tile: col block
            # i*2048:(i+1)*2048 = dim-rows i*128:(i+1)*128 of xT
            x_all = p1.tile([128, NDT * S], BF16, tag="xall", name="xall")
            wqb = p1.tile([128, DIM], BF16, tag="wqb", name="wqb")
            wkb = p1.tile([128, DIM], BF16, tag="wkb", name="wkb")
            wvb = p1.tile([128, DIM], BF16, tag="wvb", name="wvb")
            cosT = p1.tile([DL, S], BF16, tag="cosT", name="cosT")
            sinTs = p1.tile([DL, S], BF16, tag="sinTs", name="sinTs")

            def xs(i):
                return x_all[:, i * S:(i + 1) * S]

            # ---- input DMAs: 3 queues, priority order inside each ----
            # x arrives in 512-col-quarter bundles so the first rope
            # passes (which need only cols 0:512 / 512:1024) start as
            # early as possible; weights/tables interleaved by first use.
            x3 = x_all[:].rearrange("p (t s) -> p t s", s=S)
            xe3 = xT_e[:].rearrange("(t p) s -> p t s", p=128)
            nc.gpsimd.dma_start(wkb[:], wkB_e[:])
            nc.gpsimd.dma_start(wqb[:], wqB_e[:])
            nc.sync.dma_start(x3[:, 0:3, 0:512], xe3[:, 0:3, 0:512])
            nc.scalar.dma_start(x3[:, 3:6, 0:512], xe3[:, 3:6, 0:512])
            nc.gpsimd.dma_start(x3[:, 6:8, 0:512], xe3[:, 6:8, 0:512])
            nc.sync.dma_start(cosT[:, 0:1024], cosT_e[:, 0:1024])
            nc.scalar.dma_start(sinTs[:, 0:1024], sinTs_e[:, 0:1024])
            nc.sync.dma_start(x3[:, 0:3, 512:1024], xe3[:, 0:3, 512:1024])
            nc.scalar.dma_start(x3[:, 3:6, 512:1024], xe3[:, 3:6, 512:1024])
            nc.gpsimd.dma_start(x3[:, 6:8, 512:1024], xe3[:, 6:8, 512:1024])
            nc.gpsimd.dma_start(bq[:], bq_e[:])
            nc.gpsimd.dma_start(bk[:], bk_e[:])
            nc.sync.dma_start(x3[:, 0:3, 1024:2048], xe3[:, 0:3, 1024:2048])
            nc.scalar.dma_start(x3[:, 3:6, 1024:2048], xe3[:, 3:6, 1024:2048])
            nc.gpsimd.dma_start(x3[:, 6:8, 1024:2048], xe3[:, 6:8, 1024:2048])
            nc.gpsimd.dma_start(cosT[:, 1024:2048], cosT_e[:, 1024:2048])
            nc.gpsimd.dma_start(sinTs[:, 1024:2048], sinTs_e[:, 1024:2048])
            nc.gpsimd.dma_start(wvb[:], wvB_e[:])
            nc.sync.dma_start(bvb[:], bvb_e[:])
            nc.scalar.dma_start(wpT[:], wpT_e[:])

            # ---- init + PE warm-up ----
            nc.vector.memset(warm[:], 0.0)
            nc.vector.memset(maskAB[64:65, :], 0.0)
            nc.vector.memset(maskAB[64:65, 0:64], 1.0)
            nc.vector.memset(maskAB[64:65, 192:256], 1.0)
            nc.vector.memset(ones16[:], 1.0)
            v3 = vAB[:].rearrange("p (t c) -> p t c", c=65)  # [128, 32, 65]
            nc.vector.tensor_copy(
                v3[:, :, 64:65],
                ones16[:, 0:1].unsqueeze(2).broadcast_to((128, 32, 1)))
            wps = ps_sc.tile([128, 1024], F32, tag="sc", name="warmps")
            for i in range(12):
                nc.tensor.matmul(wps[:, (i % 2) * 512:(i % 2) * 512 + 512],
                                 warm[:, 0:128], warm[:, 0:512],
                                 start=True, stop=True)
            # preload the Exp activation table while ACT is idle
            nc.scalar.activation(warm[0:1, 0:2], warm[0:1, 0:2], EXP)

            # ---- phase 1: rope passes (512 cols each) ----
            def rope_pass(wb, bias, dest, c, swap_eng):
                cs = c * 512
                ps = ps_sm.tile([128, 512], F32, tag="sm", name="ropeps")
                for i in range(NDT):
                    nc.tensor.matmul(ps[:], wb[:, i * 128:(i + 1) * 128],
                                     xs(i)[:, cs:cs + 512],
                                     start=(i == 0), stop=(i == NDT - 1))
                qb = rtp.tile([128, 512], BF16, tag="qb", name="qb")
                nc.vector.tensor_scalar(qb[:], ps[:], bias[:, 0:1], None,
                                        op0=ADD)
                qsw = rtp.tile([128, 512], BF16, tag="qsw", name="qsw")
                for d0, s0 in ((0, 32), (32, 0), (64, 96), (96, 64)):
                    if swap_eng is nc.scalar:
                        swap_eng.copy(qsw[d0:d0 + 32, :], qb[s0:s0 + 32, :])
                    else:
                        swap_eng.tensor_copy(qsw[d0:d0 + 32, :],
                                             qb[s0:s0 + 32, :])
                t2 = rtp.tile([128, 512], BF16, tag="t2", name="t2")
                nc.vector.tensor_mul(t2[:], qsw[:], sinTs[:, cs:cs + 512])
                nc.vector.tensor_mul(dest[:, cs:cs + 512], qb[:],
                                     cosT[:, cs:cs + 512])
                nc.vector.tensor_add(dest[:, cs:cs + 512],
                                     dest[:, cs:cs + 512], t2[:])

            def v_tiles(ts_range):
                for t in ts_range:
                    ps = ps_sm.tile([128, 512], F32, tag="sm", name="vps")
                    for i in range(NDT):
                        nc.tensor.matmul(
                            ps[:, 0:128],
                            xs(i)[:, t * 128:(t + 1) * 128],
                            wvb[:, i * 128:(i + 1) * 128],
                            start=(i == 0), stop=(i == NDT - 1))
                    blk = vAB[:, t * 130:(t + 1) * 130].rearrange(
                        "p (b c) -> p b c", c=65)
                    nc.vector.tensor_add(
                        blk[:, :, 0:64],
                        ps[:, 0:128].rearrange("p (b c) -> p b c", c=64),
                        bvb[:].rearrange("p (b c) -> p b c", c=64))

            # ---- scores + exp ----
            pts = {}

            def sc_kt(pair, kt):
                cs0 = pair * 1024
                for hp, hname in ((0, "A"), (64, "B")):
                    ps = ps_sc.tile([128, 1024], F32, tag="sc", name="scps")
                    for j in range(2):
                        nc.tensor.matmul(
                            ps[:, j * 512:(j + 1) * 512],
                            k_rot[hp:hp + 64, kt * 128:(kt + 1) * 128],
                            q_rot[hp:hp + 64, cs0 + j * 512:cs0 + j * 512 + 512],
                            start=True, stop=True)
                    pt = ptp.tile([128, 1024], BF16, tag="pt", name="pt")
                    nc.scalar.activation(pt[:], ps[:], EXP)
                    pts[(pair, hname, kt)] = pt

            # ---- attn@v chains ----
            av_state = {}

            def av_open(pair, cc, wide=False):
                if wide:
                    # both chains in one [128,1024] tile from the big pool
                    # (A in the first bank, B in the second)
                    w = ps_sc.tile([128, 1024], F32, tag="sc", name="pavW")
                    av_state[(pair, cc)] = ((w, 0), (w, 512))
                else:
                    av_state[(pair, cc)] = (
                        (ps_sm.tile([128, 512], F32, tag="sm",
                                    name="pavA"), 0),
                        (ps_sm.tile([128, 512], F32, tag="sm",
                                    name="pavB"), 0))

            def av_kt(pair, cc, kt):
                pavA, pavB = av_state[(pair, cc)]
                for hb, (pav, c0), nm in ((0, pavA, "A"), (1, pavB, "B")):
                    bc = kt * 130 + hb * 65
                    nc.tensor.matmul(
                        pav[0:65, c0:c0 + 512], vAB[0:128, bc:bc + 65],
                        pts[(pair, nm, kt)][0:128, cc * 512:cc * 512 + 512],
                        start=(kt == 0), stop=(kt == NKT - 1))

            def av_evac(pair, cc, on_act=False):
                (pavA, cA), (pavB, cB) = av_state.pop((pair, cc))
                csq = (pair * 2 + cc) * 512
                # head A -> outA rows 0:64 (+ den row 64); head B values go
                # straight to outN rows 64:128 (tensor_tensor needs matched
                # input base partitions), B's den row to outB row 64.
                if on_act:
                    nc.scalar.copy(outA[:, csq:csq + 512],
                                   pavA[0:65, cA:cA + 512])
                    nc.scalar.copy(outN[64:128, csq:csq + 512],
                                   pavB[0:64, cB:cB + 512])
                    nc.scalar.copy(outB[64:65, csq:csq + 512],
                                   pavB[64:65, cB:cB + 512])
                else:
                    nc.vector.tensor_copy(outA[:, csq:csq + 512],
                                          pavA[0:65, cA:cA + 512])
                    nc.vector.tensor_copy(outN[64:128, csq:csq + 512],
                                          pavB[0:64, cB:cB + 512])
                    nc.vector.tensor_copy(outB[64:65, csq:csq + 512],
                                          pavB[64:65, cB:cB + 512])

            def av_sweep(pair, cc):
                av_open(pair, cc)
                for kt in range(NKT):
                    av_kt(pair, cc, kt)
                av_evac(pair, cc)

            # ---- normalize via K=1 broadcast matmuls ----
            def norm_cc(pair, cc):
                csq = (pair * 2 + cc) * 512
                shp = ps_sc.tile([128, 1024], F32, tag="sc", name="shp")
                nc.tensor.matmul(shp[:, 0:512], maskAB[64:65, 0:128],
                                 outA[64:65, csq:csq + 512],
                                 start=True, stop=False)
                nc.tensor.matmul(shp[:, 0:512], maskAB[64:65, 128:256],
                                 outB[64:65, csq:csq + 512],
                                 start=False, stop=True)
                sh = ntp.tile([128, 512], F32, tag="sh", name="sh")
                nc.vector.reciprocal_approx_fast(sh[:], shp[:, 0:512])
                nc.vector.tensor_mul(outN[0:64, csq:csq + 512],
                                     outA[0:64, csq:csq + 512], sh[0:64, :])
                nc.vector.tensor_mul(outN[64:128, csq:csq + 512],
                                     outN[64:128, csq:csq + 512],
                                     sh[64:128, :])

            # ---- proj: one 128-seq chunk ----
            store_q = [nc.sync, nc.scalar, nc.gpsimd]
            store_n = [0]

            def proj_chunk(pair, cc, u, tail=False):
                ss = (pair * 2 + cc) * 512 + u * 128
                ps = ps_sc.tile([128, 1024], F32, tag="sc", name="prps")
                for nch in range(2):
                    nc.tensor.matmul(
                        ps[:, nch * 512:(nch + 1) * 512],
                        outN[:, ss:ss + 128],
                        wpT[:, nch * 512:(nch + 1) * 512],
                        start=True, stop=True)
                ysb = ysbp.tile([128, 1024], BF16, tag="ysb", name="ysb")
                if tail:
                    nc.scalar.activation(
                        ysb[:], ps[:], mybir.ActivationFunctionType.Copy)
                else:
                    nc.vector.tensor_copy(ysb[:], ps[:])
                store_q[store_n[0] % 3].dma_start(out_e[ss:ss + 128, :],
                                                  ysb[:])
                store_n[0] += 1

            # ================= emission schedule =================
            # early rope passes: swaps on ACT (k) / DVE (q) — they finish
            # before the first exp; later passes: swaps on DVE.
            # k-p1 is only needed from scores kt4 (~4 exps of slack), so
            # the first q chunk-pair completes as early as possible. The
            # pre-exp swaps are split so the serial DVE rope chain stays
            # short: ACT (idle until the first exp) takes three passes,
            # DVE takes q-p1 in parallel.
            rope_pass(wkb, bk, k_rot, 0, nc.scalar)
            rope_pass(wqb, bq, q_rot, 0, nc.vector)
            rope_pass(wqb, bq, q_rot, 1, nc.vector)
            rope_pass(wkb, bk, k_rot, 1, nc.scalar)

            for kt in range(8):
                sc_kt(0, kt)
            rope_pass(wkb, bk, k_rot, 2, nc.vector)
            rope_pass(wkb, bk, k_rot, 3, nc.vector)
            for kt in range(8, 12):
                sc_kt(0, kt)
            rope_pass(wqb, bq, q_rot, 2, nc.vector)
            for kt in range(12, 16):
                sc_kt(0, kt)
            rope_pass(wqb, bq, q_rot, 3, nc.vector)
            v_tiles(range(NKT))
            p1_cm.__exit__(None, None, None)

            # pair-0 cc0 attn@v rides the pair-0 exp stream
            av_sweep(0, 0)
            # kick off pair-1 scores so ACT never idles at the pair
            # boundary; drain pair-0 cc1 (not exp-gated, fast) after
            sc_kt(1, 0)
            sc_kt(1, 1)
            norm_cc(0, 0)
            av_open(0, 1)
            for kt in range(NKT):
                av_kt(0, 1, kt)
            av_evac(0, 1)
            norm_cc(0, 1)
            # pair-1: cc0's chain pair rides the exp stream; pair-0 proj
            # chunks fill the PE gaps (transiently borrowing the third
            # score PSUM buffer)
            av_open(1, 0)
            for kt in range(2, 16):
                sc_kt(1, kt)
                av_kt(1, 0, kt - 2)
                if kt == 9:
                    # cc1's chain pair opens mid-stream in a wide big-pool
                    # tile (slot frees ~exp(1,8) via the pool cycle): it
                    # catches up at 2 kts per exp-pair and rides the last
                    # exps, so almost no attn@v work is left post-exp
                    av_open(1, 1, wide=True)
                if kt >= 10:
                    av_kt(1, 1, 2 * (kt - 10))
                    av_kt(1, 1, 2 * (kt - 10) + 1)
                if kt % 2 == 0:
                    proj_chunk(0, (kt - 2) // 8, ((kt - 2) // 2) % 4)
            proj_chunk(0, 1, 3)
            for kt in range(14, 16):
                av_kt(1, 0, kt)
            for kt in range(12, 16):
                av_kt(1, 1, kt)
            av_evac(1, 0)
            norm_cc(1, 0)
            av_evac(1, 1, on_act=True)
            proj_chunk(1, 0, 0)
            proj_chunk(1, 0, 1, tail=True)
            norm_cc(1, 1)
            proj_chunk(1, 0, 2)
            proj_chunk(1, 0, 3, tail=True)
            for u in range(4):
                proj_chunk(1, 1, u, tail=(u % 2 == 1))
            ptp_cm.__exit__(None, None, None)

    nc.compile()
    return nc


def make_in_maps(x, sin, cos, W_qkv, b_qkv):
    x = np.asarray(x, np.float32)
    sin = np.asarray(sin, np.float32)
    cos = np.asarray(cos, np.float32)
    W_qkv = np.asarray(W_qkv, np.float32)
    b_qkv = np.asarray(b_qkv, np.float32)

    xT = np.ascontiguousarray(x.T).astype(ml_dtypes.bfloat16)
    # sin/cos halves are duplicated (ang = concat([ang, ang])); rows are
    # [h0 d0:32, h0 d32:64, h1 d0:32, h1 d32:64] -> 4x tile of the
    # first-half columns works for cos. The rotate-half sign pattern is
    # [-s, +s, -s, +s] per 32-row block.
    cosT = np.ascontiguousarray(np.tile(cos[:, :32].T, (4, 1))).astype(
        ml_dtypes.bfloat16)
    sin32 = sin[:, :32].T
    sinTs = np.ascontiguousarray(
        np.concatenate([-sin32, sin32, -sin32, sin32], 0)).astype(
            ml_dtypes.bfloat16)

    scale = 1.0 / np.sqrt(np.float32(D))
    Wq = W_qkv[0:DIM] * scale
    Wk = W_qkv[DIM:2 * DIM]
    Wv = W_qkv[2 * DIM:3 * DIM]
    bq_full = b_qkv[0:DIM] * scale
    bk_full = b_qkv[DIM:2 * DIM]
    bv_full = b_qkv[2 * DIM:3 * DIM]

    def bundle(wT):
        # [1024, 128] lhsT layout -> [128, 8*128] col-block bundle
        return np.ascontiguousarray(
            wT.reshape(NDT, 128, DL).transpose(1, 0, 2).reshape(128, DIM)
        ).astype(ml_dtypes.bfloat16)

    in_maps = []
    for core in range(N_CORES):
        h0, h1 = 2 * core, 2 * core + 1

        def head_rows(W):
            return np.concatenate([W[h0 * D:(h0 + 1) * D],
                                   W[h1 * D:(h1 + 1) * D]], 0)

        wq_c = head_rows(Wq)
        wk_c = head_rows(Wk)
        wv_c = head_rows(Wv)
        bq_c = head_rows(bq_full[:, None])
        bk_c = head_rows(bk_full[:, None])
        bv_row = head_rows(bv_full[:, None])[:, 0]
        bvb_c = np.broadcast_to(bv_row[None, :], (DL, DL))
        in_maps.append({
            "xT": xT,
            "wqB": bundle(np.ascontiguousarray(wq_c.T)),
            "wkB": bundle(np.ascontiguousarray(wk_c.T)),
            "wvB": bundle(np.ascontiguousarray(wv_c.T)),
            "cosT": cosT,
            "sinTs": sinTs,
            "bq": np.ascontiguousarray(bq_c),
            "bk": np.ascontiguousarray(bk_c),
            "bvb": np.ascontiguousarray(bvb_c),
        })
    return in_maps


def add_wp(in_maps, W_proj):
    W_proj = np.asarray(W_proj, np.float32)
    for core in range(N_CORES):
        cols = slice(core * DL, (core + 1) * DL)
        in_maps[core]["wpT"] = np.ascontiguousarray(
            W_proj[:, cols].T).astype(ml_dtypes.bfloat16)
    return in_maps


_NC_CACHE = {}


def kernel(x, sin, cos, W_qkv, b_qkv, W_proj, b_proj):
    if "nc" not in _NC_CACHE:
        _NC_CACHE["nc"] = build()
    nc = _NC_CACHE["nc"]
    in_maps = add_wp(make_in_maps(x, sin, cos, W_qkv, b_qkv), W_proj)
    res = bass_utils.run_bass_kernel_spmd(
        nc, in_maps, core_ids=list(range(N_CORES)))
    y = np.zeros((S, DIM), np.float64)
    for core in range(N_CORES):
        y += res.results[core]["out"].astype(np.float64)
    y += np.asarray(b_proj, np.float32)[None, :].astype(np.float64)
    return y.astype(np.float32)


# revision 42
# speedup vs baseline: 1.0689x; 1.0689x over previous
"""Trainium2 Bass kernel: full-sequence multi-head attention
(S=2048, DIM=1024, H=16, D=64) sharded across 8 NeuronCores with
tensor parallelism on heads (2 heads per core), zero device collectives.

v3 — ACT(exp)-bottleneck-centric schedule. Per-core program:

  phase 1 (8 rope passes of 512 cols): qkvT matmuls (PE, K=128) ->
    DVE evac+bias (bf16) -> rotate-half swap copies (ACT for the first
    four passes, which finish before any exp; DVE for the rest) ->
    DVE mul/mul/add with cos / signed-sin tables -> q_rot/k_rot.
    x arrives via 3 bundled wide DMAs per column-half (one per queue)
    into a single wide tile, so transfers start early and saturate HBM.
    v is computed as [seq, d] tiles with the ones-column trick (vAB).
  scores: per k-tile, two concurrent 64-row matmuls (head A rows 0:63,
    head B rows 64:127) -> [128,1024] PSUM -> ACT exp -> bf16 pt tile.
    ACT does nothing else between the first and last exp.
  attn@v: K=128 single-accumulation chains, one per (pair, cc, head):
    16 matmuls N=512 into one PSUM bank. Pair-0 cc0 rides the pair-0
    exp stream; pair-1's four chains ride the pair-1 exp stream
    together (4 PSUM banks) so almost nothing is left after the last
    exp. Head A evacs to outA (+den row), head B directly into outN
    rows 64:128 (+den row to outB) to satisfy the tensor_tensor
    matched-base-partition rule.
  normalize: two K=1 broadcast matmuls (mask row x denom row) ->
    [128,512] PSUM -> DVE fast reciprocal -> two DVE muls -> outN.
  proj: per 128-seq chunk, 2 matmuls vs wpT -> [128,1024] PSUM ->
    evac (DVE; ACT for the post-exp tail) -> DMA out bf16 partials.
  PE is pre-warmed with dummy matmuls so HAM reaches 8/8 before the
  first real matmul.

Host: y = sum_c y_c + b_proj (float64 accumulate). Host-side prep:
x pre-transposed, per-core head-sliced weights pre-transposed/bundled,
1/sqrt(D) folded into W_q/b_q, RoPE tables expanded to [128, S] with
the rotate-half sign folded into the sin table.
"""

import sys

if "/opt/trn_rl_repo" not in sys.path:
    sys.path.insert(0, "/opt/trn_rl_repo")

import numpy as np
import ml_dtypes

from concourse import bass, bacc, tile, bass_utils

mybir = bass.mybir
F32 = mybir.dt.float32
BF16 = mybir.dt.bfloat16
EXP = mybir.ActivationFunctionType.Exp
ADD = mybir.AluOpType.add
MULT = mybir.AluOpType.mult

S, DIM, H, D = 2048, 1024, 16, 64
N_CORES = 8
HPC = 2  # heads per core
DL = HPC * D  # local head dims = 128
NKT = S // 128  # 16 k tiles
NDT = DIM // 128  # 8 contraction tiles for qkv


def build():
    nc = bacc.Bacc("TRN2", target_bir_lowering=False, debug=False,
                   num_devices=N_CORES)

    xT_e = nc.dram_tensor("xT", [DIM, S], BF16, kind="ExternalInput").ap()
    # weight bundles: col block i holds dim-rows i*128:(i+1)*128 of W*T
    wqB_e = nc.dram_tensor("wqB", [128, DIM], BF16, kind="ExternalInput").ap()
    wkB_e = nc.dram_tensor("wkB", [128, DIM], BF16, kind="ExternalInput").ap()
    wvB_e = nc.dram_tensor("wvB", [128, DIM], BF16, kind="ExternalInput").ap()
    cosT_e = nc.dram_tensor("cosT", [DL, S], BF16, kind="ExternalInput").ap()
    sinTs_e = nc.dram_tensor("sinTs", [DL, S], BF16, kind="ExternalInput").ap()
    wpT_e = nc.dram_tensor("wpT", [DL, DIM], BF16, kind="ExternalInput").ap()
    bq_e = nc.dram_tensor("bq", [DL, 1], F32, kind="ExternalInput").ap()
    bk_e = nc.dram_tensor("bk", [DL, 1], F32, kind="ExternalInput").ap()
    bvb_e = nc.dram_tensor("bvb", [DL, DL], F32, kind="ExternalInput").ap()
    out_e = nc.dram_tensor("out", [S, DIM], BF16, kind="ExternalOutput").ap()

    with tile.TileContext(nc) as tc:
        with tc.tile_pool(name="persist", bufs=1) as pp, \
             tc.tile_pool(name="ps_sc", bufs=3, space="PSUM") as ps_sc, \
             tc.tile_pool(name="ps_sm", bufs=2, space="PSUM") as ps_sm, \
             tc.tile_pool(name="rope_t", bufs=6) as rtp, \
             tc.tile_pool(name="norm_t", bufs=4) as ntp, \
             tc.tile_pool(name="ysb", bufs=4) as ysbp:
            q_rot = pp.tile([128, S], BF16, tag="q_rot", name="q_rot")
            k_rot = pp.tile([128, S], BF16, tag="k_rot", name="k_rot")
            # per k-tile block of 130 cols: [vA(64) | 1 | vB(64) | 1]
            vAB = pp.tile([128, NKT * 130], BF16, tag="vAB", name="vAB")
            outA = pp.tile([65, S], BF16, tag="outA", name="outA")
            outB = pp.tile([65, S], BF16, tag="outB", name="outB")
            outN = pp.tile([128, S], BF16, tag="outN", name="outN")
            wpT = pp.tile([DL, DIM], BF16, tag="wpT", name="wpT")
            bq = pp.tile([DL, 1], F32, tag="bq", name="bq")
            bk = pp.tile([DL, 1], F32, tag="bk", name="bk")
            bvb = pp.tile([DL, DL], F32, tag="bvb", name="bvb")
            ones16 = pp.tile([128, 16], F32, tag="ones16", name="ones16")
            # broadcast masks live on partition 64 (same base partition as
            # the denominator rows in outA/outB): cols 0:128 = head-A mask,
            # cols 128:256 = head-B mask
            maskAB = pp.tile([65, 256], BF16, tag="maskAB", name="maskAB")
            warm = pp.tile([128, 512], BF16, tag="warm", name="warm")
            # pt pool opened BEFORE the phase-1 input pool so p1 can be
            # released mid-kernel (pools release in LIFO order)
            ptp_cm = tc.tile_pool(name="pt", bufs=44)
            ptp = ptp_cm.__enter__()
            p1_cm = tc.tile_pool(name="p1in", bufs=1)
            p1 = p1_cm.__enter__()
            # all 8 qkv contraction tiles in one wide tile: col block
            # i*2048:(i+1)*2048 = dim-rows i*128:(i+1)*128 of xT
            x_all = p1.tile([128, NDT * S], BF16, tag="xall", name="xall")
            wqb = p1.tile([128, DIM], BF16, tag="wqb", name="wqb")
            wkb = p1.tile([128, DIM], BF16, tag="wkb", name="wkb")
            wvb = p1.tile([128, DIM], BF16, tag="wvb", name="wvb")
            cosT = p1.tile([DL, S], BF16, tag="cosT", name="cosT")
            sinTs = p1.tile([DL, S], BF16, tag="sinTs", name="sinTs")

            def xs(i):
                return x_all[:, i * S:(i + 1) * S]

            # ---- input DMAs: 3 queues, priority order inside each ----
            # x arrives in 512-col-quarter bundles so the first rope
            # passes (which need only cols 0:512 / 512:1024) start as
            # early as possible; weights/tables interleaved by first use.
            x3 = x_all[:].rearrange("p (t s) -> p t s", s=S)
            xe3 = xT_e[:].rearrange("(t p) s -> p t s", p=128)
            nc.gpsimd.dma_start(wkb[:], wkB_e[:])
            nc.gpsimd.dma_start(wqb[:], wqB_e[:])
            nc.sync.dma_start(x3[:, 0:3, 0:512], xe3[:, 0:3, 0:512])
            nc.scalar.dma_start(x3[:, 3:6, 0:512], xe3[:, 3:6, 0:512])
            nc.gpsimd.dma_start(x3[:, 6:8, 0:512], xe3[:, 6:8, 0:512])
            nc.sync.dma_start(cosT[:, 0:1024], cosT_e[:, 0:1024])
            nc.scalar.dma_start(sinTs[:, 0:1024], sinTs_e[:, 0:1024])
            nc.sync.dma_start(x3[:, 0:3, 512:1024], xe3[:, 0:3, 512:1024])
            nc.scalar.dma_start(x3[:, 3:6, 512:1024], xe3[:, 3:6, 512:1024])
            nc.gpsimd.dma_start(x3[:, 6:8, 512:1024], xe3[:, 6:8, 512:1024])
            nc.gpsimd.dma_start(bq[:], bq_e[:])
            nc.gpsimd.dma_start(bk[:], bk_e[:])
            nc.sync.dma_start(x3[:, 0:3, 1024:2048], xe3[:, 0:3, 1024:2048])
            nc.scalar.dma_start(x3[:, 3:6, 1024:2048], xe3[:, 3:6, 1024:2048])
            nc.gpsimd.dma_start(x3[:, 6:8, 1024:2048], xe3[:, 6:8, 1024:2048])
            nc.gpsimd.dma_start(cosT[:, 1024:2048], cosT_e[:, 1024:2048])
            nc.gpsimd.dma_start(sinTs[:, 1024:2048], sinTs_e[:, 1024:2048])
            nc.gpsimd.dma_start(wvb[:], wvB_e[:])
            nc.sync.dma_start(bvb[:], bvb_e[:])
            nc.scalar.dma_start(wpT[:], wpT_e[:])

            # ---- init + PE warm-up ----
            nc.vector.memset(warm[:], 0.0)
            nc.vector.memset(maskAB[64:65, :], 0.0)
            nc.vector.memset(maskAB[64:65, 0:64], 1.0)
            nc.vector.memset(maskAB[64:65, 192:256], 1.0)
            nc.vector.memset(ones16[:], 1.0)
            v3 = vAB[:].rearrange("p (t c) -> p t c", c=65)  # [128, 32, 65]
            nc.vector.tensor_copy(
                v3[:, :, 64:65],
                ones16[:, 0:1].unsqueeze(2).broadcast_to((128, 32, 1)))
            wps = ps_sc.tile([128, 1024], F32, tag="sc", name="warmps")
            for i in range(12):
                nc.tensor.matmul(wps[:, (i % 2) * 512:(i % 2) * 512 + 512],
                                 warm[:, 0:128], warm[:, 0:512],
                                 start=True, stop=True)
            # preload the Exp activation table while ACT is idle
            nc.scalar.activation(warm[0:1, 0:2], warm[0:1, 0:2], EXP)

            # ---- phase 1: rope passes (512 cols each) ----
            def rope_pass(wb, bias, dest, c, swap_eng):
                cs = c * 512
                ps = ps_sm.tile([128, 512], F32, tag="sm", name="ropeps")
                for i in range(NDT):
                    nc.tensor.matmul(ps[:], wb[:, i * 128:(i + 1) * 128],
                                     xs(i)[:, cs:cs + 512],
                                     start=(i == 0), stop=(i == NDT - 1))
                qb = rtp.tile([128, 512], BF16, tag="qb", name="qb")
                nc.vector.tensor_scalar(qb[:], ps[:], bias[:, 0:1], None,
                                        op0=ADD)
                qsw = rtp.tile([128, 512], BF16, tag="qsw", name="qsw")
                for d0, s0 in ((0, 32), (32, 0), (64, 96), (96, 64)):
                    if swap_eng is nc.scalar:
                        swap_eng.copy(qsw[d0:d0 + 32, :], qb[s0:s0 + 32, :])
                    else:
                        swap_eng.tensor_copy(qsw[d0:d0 + 32, :],
                                             qb[s0:s0 + 32, :])
                t2 = rtp.tile([128, 512], BF16, tag="t2", name="t2")
                nc.vector.tensor_mul(t2[:], qsw[:], sinTs[:, cs:cs + 512])
                nc.vector.tensor_mul(dest[:, cs:cs + 512], qb[:],
                                     cosT[:, cs:cs + 512])
                nc.vector.tensor_add(dest[:, cs:cs + 512],
                                     dest[:, cs:cs + 512], t2[:])

            def v_tiles(ts_range):
                for t in ts_range:
                    ps = ps_sm.tile([128, 512], F32, tag="sm", name="vps")
                    for i in range(NDT):
                        nc.tensor.matmul(
                            ps[:, 0:128],
                            xs(i)[:, t * 128:(t + 1) * 128],
                            wvb[:, i * 128:(i + 1) * 128],
                            start=(i == 0), stop=(i == NDT - 1))
                    blk = vAB[:, t * 130:(t + 1) * 130].rearrange(
                        "p (b c) -> p b c", c=65)
                    nc.vector.tensor_add(
                        blk[:, :, 0:64],
                        ps[:, 0:128].rearrange("p (b c) -> p b c", c=64),
                        bvb[:].rearrange("p (b c) -> p b c", c=64))

            # ---- scores + exp ----
            pts = {}

            def sc_kt(pair, kt):
                cs0 = pair * 1024
                for hp, hname in ((0, "A"), (64, "B")):
                    ps = ps_sc.tile([128, 1024], F32, tag="sc", name="scps")
                    for j in range(2):
                        nc.tensor.matmul(
                            ps[:, j * 512:(j + 1) * 512],
                            k_rot[hp:hp + 64, kt * 128:(kt + 1) * 128],
                            q_rot[hp:hp + 64, cs0 + j * 512:cs0 + j * 512 + 512],
                            start=True, stop=True)
                    pt = ptp.tile([128, 1024], BF16, tag="pt", name="pt")
                    nc.scalar.activation(pt[:], ps[:], EXP)
                    pts[(pair, hname, kt)] = pt

            # ---- attn@v chains ----
            av_state = {}

            def av_open(pair, cc, wide=False):
                if wide:
                    # both chains in one [128,1024] tile from the big pool
                    # (A in the first bank, B in the second)
                    w = ps_sc.tile([128, 1024], F32, tag="sc", name="pavW")
                    av_state[(pair, cc)] = ((w, 0), (w, 512))
                else:
                    av_state[(pair, cc)] = (
                        (ps_sm.tile([128, 512], F32, tag="sm",
                                    name="pavA"), 0),
                        (ps_sm.tile([128, 512], F32, tag="sm",
                                    name="pavB"), 0))

            def av_kt(pair, cc, kt):
                pavA, pavB = av_state[(pair, cc)]
                for hb, (pav, c0), nm in ((0, pavA, "A"), (1, pavB, "B")):
                    bc = kt * 130 + hb * 65
                    nc.tensor.matmul(
                        pav[0:65, c0:c0 + 512], vAB[0:128, bc:bc + 65],
                        pts[(pair, nm, kt)][0:128, cc * 512:cc * 512 + 512],
                        start=(kt == 0), stop=(kt == NKT - 1))

            def av_evac(pair, cc, on_act=False):
                (pavA, cA), (pavB, cB) = av_state.pop((pair, cc))
                csq = (pair * 2 + cc) * 512
                # head A -> outA rows 0:64 (+ den row 64); head B values go
                # straight to outN rows 64:128 (tensor_tensor needs matched
                # input base partitions), B's den row to outB row 64.
                if on_act:
                    nc.scalar.copy(outA[:, csq:csq + 512],
                                   pavA[0:65, cA:cA + 512])
                    nc.scalar.copy(outN[64:128, csq:csq + 512],
                                   pavB[0:64, cB:cB + 512])
                    nc.scalar.copy(outB[64:65, csq:csq + 512],
                                   pavB[64:65, cB:cB + 512])
                else:
                    nc.vector.tensor_copy(outA[:, csq:csq + 512],
                                          pavA[0:65, cA:cA + 512])
                    nc.vector.tensor_copy(outN[64:128, csq:csq + 512],
                                          pavB[0:64, cB:cB + 512])
                    nc.vector.tensor_copy(outB[64:65, csq:csq + 512],
                                          pavB[64:65, cB:cB + 512])

            def av_sweep(pair, cc):
                av_open(pair, cc)
                for kt in range(NKT):
                    av_kt(pair, cc, kt)
                av_evac(pair, cc)

            # ---- normalize via K=1 broadcast matmuls ----
            def norm_cc(pair, cc):
                csq = (pair * 2 + cc) * 512
                shp = ps_sc.tile([128, 1024], F32, tag="sc", name="shp")
                nc.tensor.matmul(shp[:, 0:512], maskAB[64:65, 0:128],
                                 outA[64:65, csq:csq + 512],
                                 start=True, stop=False)
                nc.tensor.matmul(shp[:, 0:512], maskAB[64:65, 128:256],
                                 outB[64:65, csq:csq + 512],
                                 start=False, stop=True)
                sh = ntp.tile([128, 512], F32, tag="sh", name="sh")
                nc.vector.reciprocal_approx_fast(sh[:], shp[:, 0:512])
                nc.vector.tensor_mul(outN[0:64, csq:csq + 512],
                                     outA[0:64, csq:csq + 512], sh[0:64, :])
                nc.vector.tensor_mul(outN[64:128, csq:csq + 512],
                                     outN[64:128, csq:csq + 512],
                                     sh[64:128, :])

            # ---- proj: one 128-seq chunk ----
            store_q = [nc.sync, nc.scalar, nc.gpsimd]
            store_n = [0]

            def proj_chunk(pair, cc, u, tail=False):
                ss = (pair * 2 + cc) * 512 + u * 128
                ps = ps_sc.tile([128, 1024], F32, tag="sc", name="prps")
                for nch in range(2):
                    nc.tensor.matmul(
                        ps[:, nch * 512:(nch + 1) * 512],
                        outN[:, ss:ss + 128],
                        wpT[:, nch * 512:(nch + 1) * 512],
                        start=True, stop=True)
                ysb = ysbp.tile([128, 1024], BF16, tag="ysb", name="ysb")
                if tail:
                    nc.scalar.activation(
                        ysb[:], ps[:], mybir.ActivationFunctionType.Copy)
                else:
                    nc.vector.tensor_copy(ysb[:], ps[:])
                store_q[store_n[0] % 3].dma_start(out_e[ss:ss + 128, :],
                                                  ysb[:])
                store_n[0] += 1

            # ================= emission schedule =================
            # early rope passes: swaps on ACT (k) / DVE (q) — they finish
            # before the first exp; later passes: swaps on DVE.
            # k-p1 is only needed from scores kt4 (~4 exps of slack), so
            # the first q chunk-pair completes as early as possible. The
            # pre-exp swaps are split so the serial DVE rope chain stays
            # short: ACT (idle until the first exp) takes three passes,
            # DVE takes q-p1 in parallel.
            rope_pass(wkb, bk, k_rot, 0, nc.scalar)
            rope_pass(wqb, bq, q_rot, 0, nc.vector)
            rope_pass(wqb, bq, q_rot, 1, nc.vector)
            rope_pass(wkb, bk, k_rot, 1, nc.scalar)

            for kt in range(8):
                sc_kt(0, kt)
            rope_pass(wkb, bk, k_rot, 2, nc.vector)
            rope_pass(wkb, bk, k_rot, 3, nc.vector)
            for kt in range(8, 12):
                sc_kt(0, kt)
            rope_pass(wqb, bq, q_rot, 2, nc.vector)
            for kt in range(12, 16):
                sc_kt(0, kt)
            rope_pass(wqb, bq, q_rot, 3, nc.vector)
            v_tiles(range(NKT))
            p1_cm.__exit__(None, None, None)

            # pair-0 cc0 attn@v rides the pair-0 exp stream
            av_sweep(0, 0)
            # kick off pair-1 scores so ACT never idles at the pair
            # boundary; drain pair-0 cc1 (not exp-gated, fast) after
            sc_kt(1, 0)
            sc_kt(1, 1)
            norm_cc(0, 0)
            av_open(0, 1)
            for kt in range(NKT):
                av_kt(0, 1, kt)
            av_evac(0, 1)
            norm_cc(0, 1)
            # pair-1: cc0's chain pair rides the exp stream; pair-0 proj
            # chunks fill the PE gaps (transiently borrowing the third
            # score PSUM buffer)
            av_open(1, 0)
            for kt in range(2, 16):
                sc_kt(1, kt)
                av_kt(1, 0, kt - 2)
                if kt % 2 == 0:
                    proj_chunk(0, (kt - 2) // 8, ((kt - 2) // 2) % 4)
            proj_chunk(0, 1, 3)
            for kt in range(14, 16):
                av_kt(1, 0, kt)
            av_evac(1, 0)
            norm_cc(1, 0)
            # tail: cc1 chains in one big-pool tile, interleaved with
            # pair-1 cc0 proj chunks; evac work split across DVE and the
            # now-idle ACT engine. (Opening the cc1 chains mid-stream to
            # ride the exp tail was measured: the score-buffer steal
            # stretches the exp stream more than the tail saves.)
            av_open(1, 1, wide=True)
            for kt in range(8):
                av_kt(1, 1, kt)
            proj_chunk(1, 0, 0)
            proj_chunk(1, 0, 1, tail=True)
            for kt in range(8, 16):
                av_kt(1, 1, kt)
            proj_chunk(1, 0, 2)
            av_evac(1, 1, on_act=True)
            proj_chunk(1, 0, 3, tail=True)
            norm_cc(1, 1)
            for u in range(4):
                proj_chunk(1, 1, u, tail=(u % 2 == 1))
            ptp_cm.__exit__(None, None, None)

    nc.compile()
    return nc


def make_in_maps(x, sin, cos, W_qkv, b_qkv):
    x = np.asarray(x, np.float32)
    sin = np.asarray(sin, np.float32)
    cos = np.asarray(cos, np.float32)
    W_qkv = np.asarray(W_qkv, np.float32)
    b_qkv = np.asarray(b_qkv, np.float32)

    xT = np.ascontiguousarray(x.T).astype(ml_dtypes.bfloat16)
    # sin/cos halves are duplicated (ang = concat([ang, ang])); rows are
    # [h0 d0:32, h0 d32:64, h1 d0:32, h1 d32:64] -> 4x tile of the
    # first-half columns works for cos. The rotate-half sign pattern is
    # [-s, +s, -s, +s] per 32-row block.
    cosT = np.ascontiguousarray(np.tile(cos[:, :32].T, (4, 1))).astype(
        ml_dtypes.bfloat16)
    sin32 = sin[:, :32].T
    sinTs = np.ascontiguousarray(
        np.concatenate([-sin32, sin32, -sin32, sin32], 0)).astype(
            ml_dtypes.bfloat16)

    scale = 1.0 / np.sqrt(np.float32(D))
    Wq = W_qkv[0:DIM] * scale
    Wk = W_qkv[DIM:2 * DIM]
    Wv = W_qkv[2 * DIM:3 * DIM]
    bq_full = b_qkv[0:DIM] * scale
    bk_full = b_qkv[DIM:2 * DIM]
    bv_full = b_qkv[2 * DIM:3 * DIM]

    def bundle(wT):
        # [1024, 128] lhsT layout -> [128, 8*128] col-block bundle
        return np.ascontiguousarray(
            wT.reshape(NDT, 128, DL).transpose(1, 0, 2).reshape(128, DIM)
        ).astype(ml_dtypes.bfloat16)

    in_maps = []
    for core in range(N_CORES):
        h0, h1 = 2 * core, 2 * core + 1

        def head_rows(W):
            return np.concatenate([W[h0 * D:(h0 + 1) * D],
                                   W[h1 * D:(h1 + 1) * D]], 0)

        wq_c = head_rows(Wq)
        wk_c = head_rows(Wk)
        wv_c = head_rows(Wv)
        bq_c = head_rows(bq_full[:, None])
        bk_c = head_rows(bk_full[:, None])
        bv_row = head_rows(bv_full[:, None])[:, 0]
        bvb_c = np.broadcast_to(bv_row[None, :], (DL, DL))
        in_maps.append({
            "xT": xT,
            "wqB": bundle(np.ascontiguousarray(wq_c.T)),
            "wkB": bundle(np.ascontiguousarray(wk_c.T)),
            "wvB": bundle(np.ascontiguousarray(wv_c.T)),
            "cosT": cosT,
            "sinTs": sinTs,
            "bq": np.ascontiguousarray(bq_c),
            "bk": np.ascontiguousarray(bk_c),
            "bvb": np.ascontiguousarray(bvb_c),
        })
    return in_maps


def add_wp(in_maps, W_proj):
    W_proj = np.asarray(W_proj, np.float32)
    for core in range(N_CORES):
        cols = slice(core * DL, (core + 1) * DL)
        in_maps[core]["wpT"] = np.ascontiguousarray(
            W_proj[:, cols].T).astype(ml_dtypes.bfloat16)
    return in_maps


_NC_CACHE = {}


def kernel(x, sin, cos, W_qkv, b_qkv, W_proj, b_proj):
    if "nc" not in _NC_CACHE:
        _NC_CACHE["nc"] = build()
    nc = _NC_CACHE["nc"]
    in_maps = add_wp(make_in_maps(x, sin, cos, W_qkv, b_qkv), W_proj)
    res = bass_utils.run_bass_kernel_spmd(
        nc, in_maps, core_ids=list(range(N_CORES)))
    y = np.zeros((S, DIM), np.float64)
    for core in range(N_CORES):
        y += res.results[core]["out"].astype(np.float64)
    y += np.asarray(b_proj, np.float32)[None, :].astype(np.float64)
    return y.astype(np.float32)


# revision 45
# speedup vs baseline: 1.0964x; 1.0257x over previous
"""Trainium2 Bass kernel: full-sequence multi-head attention
(S=2048, DIM=1024, H=16, D=64) sharded across 8 NeuronCores with
tensor parallelism on heads (2 heads per core), zero device collectives.

v3 — ACT(exp)-bottleneck-centric schedule. Per-core program:

  phase 1 (8 rope passes of 512 cols): qkvT matmuls (PE, K=128) ->
    DVE evac+bias (bf16) -> rotate-half swap copies (ACT for the first
    four passes, which finish before any exp; DVE for the rest) ->
    DVE mul/mul/add with cos / signed-sin tables -> q_rot/k_rot.
    x arrives via 3 bundled wide DMAs per column-half (one per queue)
    into a single wide tile, so transfers start early and saturate HBM.
    v is computed as [seq, d] tiles with the ones-column trick (vAB).
  scores: per k-tile, two concurrent 64-row matmuls (head A rows 0:63,
    head B rows 64:127) -> [128,1024] PSUM -> ACT exp -> bf16 pt tile.
    ACT does nothing else between the first and last exp.
  attn@v: K=128 single-accumulation chains, one per (pair, cc, head):
    16 matmuls N=512 into one PSUM bank. Pair-0 cc0 rides the pair-0
    exp stream; pair-1's four chains ride the pair-1 exp stream
    together (4 PSUM banks) so almost nothing is left after the last
    exp. Head A evacs to outA (+den row), head B directly into outN
    rows 64:128 (+den row to outB) to satisfy the tensor_tensor
    matched-base-partition rule.
  normalize: two K=1 broadcast matmuls (mask row x denom row) ->
    [128,512] PSUM -> DVE fast reciprocal -> two DVE muls -> outN.
  proj: per 128-seq chunk, 2 matmuls vs wpT -> [128,1024] PSUM ->
    evac (DVE; ACT for the post-exp tail) -> DMA out bf16 partials.
  PE is pre-warmed with dummy matmuls so HAM reaches 8/8 before the
  first real matmul.

Host: y = sum_c y_c + b_proj (float64 accumulate). Host-side prep:
x pre-transposed, per-core head-sliced weights pre-transposed/bundled,
1/sqrt(D) folded into W_q/b_q, RoPE tables expanded to [128, S] with
the rotate-half sign folded into the sin table.
"""

import sys

if "/opt/trn_rl_repo" not in sys.path:
    sys.path.insert(0, "/opt/trn_rl_repo")

import numpy as np
import ml_dtypes

from concourse import bass, bacc, tile, bass_utils

mybir = bass.mybir
F32 = mybir.dt.float32
BF16 = mybir.dt.bfloat16
EXP = mybir.ActivationFunctionType.Exp
ADD = mybir.AluOpType.add
MULT = mybir.AluOpType.mult

S, DIM, H, D = 2048, 1024, 16, 64
N_CORES = 8
HPC = 2  # heads per core
DL = HPC * D  # local head dims = 128
NKT = S // 128  # 16 k tiles
NDT = DIM // 128  # 8 contraction tiles for qkv


def build():
    nc = bacc.Bacc("TRN2", target_bir_lowering=False, debug=False,
                   num_devices=N_CORES)

    xT_e = nc.dram_tensor("xT", [DIM, S], BF16, kind="ExternalInput").ap()
    # weight bundles: col block i holds dim-rows i*128:(i+1)*128 of W*T
    wqB_e = nc.dram_tensor("wqB", [128, DIM], BF16, kind="ExternalInput").ap()
    wkB_e = nc.dram_tensor("wkB", [128, DIM], BF16, kind="ExternalInput").ap()
    wvB_e = nc.dram_tensor("wvB", [128, DIM], BF16, kind="ExternalInput").ap()
    cosT_e = nc.dram_tensor("cosT", [DL, S], BF16, kind="ExternalInput").ap()
    sinTs_e = nc.dram_tensor("sinTs", [DL, S], BF16, kind="ExternalInput").ap()
    wpT_e = nc.dram_tensor("wpT", [DL, DIM], BF16, kind="ExternalInput").ap()
    bq_e = nc.dram_tensor("bq", [DL, 1], F32, kind="ExternalInput").ap()
    bk_e = nc.dram_tensor("bk", [DL, 1], F32, kind="ExternalInput").ap()
    bvb_e = nc.dram_tensor("bvb", [DL, DL], F32, kind="ExternalInput").ap()
    out_e = nc.dram_tensor("out", [S, DIM], BF16, kind="ExternalOutput").ap()

    with tile.TileContext(nc) as tc:
        with tc.tile_pool(name="persist", bufs=1) as pp, \
             tc.tile_pool(name="ps_sc", bufs=3, space="PSUM") as ps_sc, \
             tc.tile_pool(name="ps_sm", bufs=2, space="PSUM") as ps_sm, \
             tc.tile_pool(name="rope_t", bufs=6) as rtp, \
             tc.tile_pool(name="norm_t", bufs=4) as ntp, \
             tc.tile_pool(name="ysb", bufs=4) as ysbp:
            q_rot = pp.tile([128, S], BF16, tag="q_rot", name="q_rot")
            k_rot = pp.tile([128, S], BF16, tag="k_rot", name="k_rot")
            # per k-tile block of 130 cols: [vA(64) | 1 | vB(64) | 1]
            vAB = pp.tile([128, NKT * 130], BF16, tag="vAB", name="vAB")
            outA = pp.tile([65, S], BF16, tag="outA", name="outA")
            outB = pp.tile([65, S], BF16, tag="outB", name="outB")
            outN = pp.tile([128, S], BF16, tag="outN", name="outN")
            wpT = pp.tile([DL, DIM], BF16, tag="wpT", name="wpT")
            bq = pp.tile([DL, 1], F32, tag="bq", name="bq")
            bk = pp.tile([DL, 1], F32, tag="bk", name="bk")
            bvb = pp.tile([DL, DL], F32, tag="bvb", name="bvb")
            ones16 = pp.tile([128, 16], F32, tag="ones16", name="ones16")
            # broadcast masks live on partition 64 (same base partition as
            # the denominator rows in outA/outB): cols 0:128 = head-A mask,
            # cols 128:256 = head-B mask
            maskAB = pp.tile([65, 256], BF16, tag="maskAB", name="maskAB")
            warm = pp.tile([128, 512], BF16, tag="warm", name="warm")
            # pt pool opened BEFORE the phase-1 input pool so p1 can be
            # released mid-kernel (pools release in LIFO order)
            ptp_cm = tc.tile_pool(name="pt", bufs=44)
            ptp = ptp_cm.__enter__()
            p1_cm = tc.tile_pool(name="p1in", bufs=1)
            p1 = p1_cm.__enter__()
            # all 8 qkv contraction tiles in one wide tile: col block
            # i*2048:(i+1)*2048 = dim-rows i*128:(i+1)*128 of xT
            x_all = p1.tile([128, NDT * S], BF16, tag="xall", name="xall")
            wqb = p1.tile([128, DIM], BF16, tag="wqb", name="wqb")
            wkb = p1.tile([128, DIM], BF16, tag="wkb", name="wkb")
            wvb = p1.tile([128, DIM], BF16, tag="wvb", name="wvb")
            cosT = p1.tile([DL, S], BF16, tag="cosT", name="cosT")
            sinTs = p1.tile([DL, S], BF16, tag="sinTs", name="sinTs")

            def xs(i):
                return x_all[:, i * S:(i + 1) * S]

            # ---- input DMAs: 3 queues, priority order inside each ----
            # x arrives in 512-col-quarter bundles so the first rope
            # passes (which need only cols 0:512 / 512:1024) start as
            # early as possible; weights/tables interleaved by first use.
            x3 = x_all[:].rearrange("p (t s) -> p t s", s=S)
            xe3 = xT_e[:].rearrange("(t p) s -> p t s", p=128)
            nc.gpsimd.dma_start(wkb[:], wkB_e[:])
            nc.gpsimd.dma_start(wqb[:], wqB_e[:])
            nc.sync.dma_start(x3[:, 0:3, 0:512], xe3[:, 0:3, 0:512])
            nc.scalar.dma_start(x3[:, 3:6, 0:512], xe3[:, 3:6, 0:512])
            nc.gpsimd.dma_start(x3[:, 6:8, 0:512], xe3[:, 6:8, 0:512])
            nc.sync.dma_start(cosT[:, 0:1024], cosT_e[:, 0:1024])
            nc.scalar.dma_start(sinTs[:, 0:1024], sinTs_e[:, 0:1024])
            nc.sync.dma_start(x3[:, 0:3, 512:1024], xe3[:, 0:3, 512:1024])
            nc.scalar.dma_start(x3[:, 3:6, 512:1024], xe3[:, 3:6, 512:1024])
            nc.gpsimd.dma_start(x3[:, 6:8, 512:1024], xe3[:, 6:8, 512:1024])
            nc.gpsimd.dma_start(bq[:], bq_e[:])
            nc.gpsimd.dma_start(bk[:], bk_e[:])
            nc.sync.dma_start(x3[:, 0:3, 1024:2048], xe3[:, 0:3, 1024:2048])
            nc.scalar.dma_start(x3[:, 3:6, 1024:2048], xe3[:, 3:6, 1024:2048])
            nc.gpsimd.dma_start(x3[:, 6:8, 1024:2048], xe3[:, 6:8, 1024:2048])
            nc.gpsimd.dma_start(cosT[:, 1024:2048], cosT_e[:, 1024:2048])
            nc.gpsimd.dma_start(sinTs[:, 1024:2048], sinTs_e[:, 1024:2048])
            nc.gpsimd.dma_start(wvb[:], wvB_e[:])
            nc.sync.dma_start(bvb[:], bvb_e[:])
            nc.scalar.dma_start(wpT[:], wpT_e[:])

            # ---- init + PE warm-up ----
            # init work on GpSimd (all SBUF) keeps the serial DVE rope
            # chain in the head as short as possible; only the warm tile
            # (needed immediately by the PE warm-up) stays on DVE
            nc.vector.memset(warm[:], 0.0)
            nc.gpsimd.memset(maskAB[64:65, :], 0.0)
            nc.gpsimd.memset(maskAB[64:65, 0:64], 1.0)
            nc.gpsimd.memset(maskAB[64:65, 192:256], 1.0)
            nc.gpsimd.memset(ones16[:], 1.0)
            v3 = vAB[:].rearrange("p (t c) -> p t c", c=65)  # [128, 32, 65]
            nc.gpsimd.tensor_copy(
                v3[:, :, 64:65],
                ones16[:, 0:1].unsqueeze(2).broadcast_to((128, 32, 1)))
            wps = ps_sc.tile([128, 1024], F32, tag="sc", name="warmps")
            for i in range(12):
                nc.tensor.matmul(wps[:, (i % 2) * 512:(i % 2) * 512 + 512],
                                 warm[:, 0:128], warm[:, 0:512],
                                 start=True, stop=True)
            # preload the Exp activation table while ACT is idle
            nc.scalar.activation(warm[0:1, 0:2], warm[0:1, 0:2], EXP)

            # ---- phase 1: rope passes (512 cols each) ----
            def rope_pass(wb, bias, dest, c, swap_eng):
                cs = c * 512
                ps = ps_sm.tile([128, 512], F32, tag="sm", name="ropeps")
                for i in range(NDT):
                    nc.tensor.matmul(ps[:], wb[:, i * 128:(i + 1) * 128],
                                     xs(i)[:, cs:cs + 512],
                                     start=(i == 0), stop=(i == NDT - 1))
                qb = rtp.tile([128, 512], BF16, tag="qb", name="qb")
                nc.vector.tensor_scalar(qb[:], ps[:], bias[:, 0:1], None,
                                        op0=ADD)
                qsw = rtp.tile([128, 512], BF16, tag="qsw", name="qsw")
                for d0, s0 in ((0, 32), (32, 0), (64, 96), (96, 64)):
                    if swap_eng is nc.scalar:
                        swap_eng.copy(qsw[d0:d0 + 32, :], qb[s0:s0 + 32, :])
                    else:
                        swap_eng.tensor_copy(qsw[d0:d0 + 32, :],
                                             qb[s0:s0 + 32, :])
                t2 = rtp.tile([128, 512], BF16, tag="t2", name="t2")
                nc.vector.tensor_mul(t2[:], qsw[:], sinTs[:, cs:cs + 512])
                nc.vector.tensor_mul(dest[:, cs:cs + 512], qb[:],
                                     cosT[:, cs:cs + 512])
                nc.vector.tensor_add(dest[:, cs:cs + 512],
                                     dest[:, cs:cs + 512], t2[:])

            def v_tiles(ts_range):
                for t in ts_range:
                    ps = ps_sm.tile([128, 512], F32, tag="sm", name="vps")
                    for i in range(NDT):
                        nc.tensor.matmul(
                            ps[:, 0:128],
                            xs(i)[:, t * 128:(t + 1) * 128],
                            wvb[:, i * 128:(i + 1) * 128],
                            start=(i == 0), stop=(i == NDT - 1))
                    blk = vAB[:, t * 130:(t + 1) * 130].rearrange(
                        "p (b c) -> p b c", c=65)
                    nc.vector.tensor_add(
                        blk[:, :, 0:64],
                        ps[:, 0:128].rearrange("p (b c) -> p b c", c=64),
                        bvb[:].rearrange("p (b c) -> p b c", c=64))

            # ---- scores + exp ----
            pts = {}

            def sc_kt(pair, kt):
                cs0 = pair * 1024
                for hp, hname in ((0, "A"), (64, "B")):
                    ps = ps_sc.tile([128, 1024], F32, tag="sc", name="scps")
                    for j in range(2):
                        nc.tensor.matmul(
                            ps[:, j * 512:(j + 1) * 512],
                            k_rot[hp:hp + 64, kt * 128:(kt + 1) * 128],
                            q_rot[hp:hp + 64, cs0 + j * 512:cs0 + j * 512 + 512],
                            start=True, stop=True)
                    pt = ptp.tile([128, 1024], BF16, tag="pt", name="pt")
                    nc.scalar.activation(pt[:], ps[:], EXP)
                    pts[(pair, hname, kt)] = pt

            # ---- attn@v chains ----
            av_state = {}

            def av_open(pair, cc, wide=False):
                if wide:
                    # both chains in one [128,1024] tile from the big pool
                    # (A in the first bank, B in the second)
                    w = ps_sc.tile([128, 1024], F32, tag="sc", name="pavW")
                    av_state[(pair, cc)] = ((w, 0), (w, 512))
                else:
                    av_state[(pair, cc)] = (
                        (ps_sm.tile([128, 512], F32, tag="sm",
                                    name="pavA"), 0),
                        (ps_sm.tile([128, 512], F32, tag="sm",
                                    name="pavB"), 0))

            def av_kt(pair, cc, kt):
                pavA, pavB = av_state[(pair, cc)]
                for hb, (pav, c0), nm in ((0, pavA, "A"), (1, pavB, "B")):
                    bc = kt * 130 + hb * 65
                    nc.tensor.matmul(
                        pav[0:65, c0:c0 + 512], vAB[0:128, bc:bc + 65],
                        pts[(pair, nm, kt)][0:128, cc * 512:cc * 512 + 512],
                        start=(kt == 0), stop=(kt == NKT - 1))

            def av_evac(pair, cc, on_act=False):
                (pavA, cA), (pavB, cB) = av_state.pop((pair, cc))
                csq = (pair * 2 + cc) * 512
                # head A -> outA rows 0:64 (+ den row 64); head B values go
                # straight to outN rows 64:128 (tensor_tensor needs matched
                # input base partitions), B's den row to outB row 64.
                if on_act:
                    nc.scalar.copy(outA[:, csq:csq + 512],
                                   pavA[0:65, cA:cA + 512])
                    nc.scalar.copy(outN[64:128, csq:csq + 512],
                                   pavB[0:64, cB:cB + 512])
                    nc.scalar.copy(outB[64:65, csq:csq + 512],
                                   pavB[64:65, cB:cB + 512])
                else:
                    nc.vector.tensor_copy(outA[:, csq:csq + 512],
                                          pavA[0:65, cA:cA + 512])
                    nc.vector.tensor_copy(outN[64:128, csq:csq + 512],
                                          pavB[0:64, cB:cB + 512])
                    nc.vector.tensor_copy(outB[64:65, csq:csq + 512],
                                          pavB[64:65, cB:cB + 512])

            def av_sweep(pair, cc):
                av_open(pair, cc)
                for kt in range(NKT):
                    av_kt(pair, cc, kt)
                av_evac(pair, cc)

            # ---- normalize via K=1 broadcast matmuls ----
            def norm_cc(pair, cc):
                csq = (pair * 2 + cc) * 512
                shp = ps_sc.tile([128, 1024], F32, tag="sc", name="shp")
                nc.tensor.matmul(shp[:, 0:512], maskAB[64:65, 0:128],
                                 outA[64:65, csq:csq + 512],
                                 start=True, stop=False)
                nc.tensor.matmul(shp[:, 0:512], maskAB[64:65, 128:256],
                                 outB[64:65, csq:csq + 512],
                                 start=False, stop=True)
                sh = ntp.tile([128, 512], F32, tag="sh", name="sh")
                nc.vector.reciprocal_approx_fast(sh[:], shp[:, 0:512])
                nc.vector.tensor_mul(outN[0:64, csq:csq + 512],
                                     outA[0:64, csq:csq + 512], sh[0:64, :])
                nc.vector.tensor_mul(outN[64:128, csq:csq + 512],
                                     outN[64:128, csq:csq + 512],
                                     sh[64:128, :])

            # ---- proj: one 128-seq chunk ----
            store_q = [nc.sync, nc.scalar, nc.gpsimd]
            store_n = [0]

            def proj_chunk(pair, cc, u, tail=False):
                ss = (pair * 2 + cc) * 512 + u * 128
                ps = ps_sc.tile([128, 1024], F32, tag="sc", name="prps")
                for nch in range(2):
                    nc.tensor.matmul(
                        ps[:, nch * 512:(nch + 1) * 512],
                        outN[:, ss:ss + 128],
                        wpT[:, nch * 512:(nch + 1) * 512],
                        start=True, stop=True)
                ysb = ysbp.tile([128, 1024], BF16, tag="ysb", name="ysb")
                if tail:
                    # post-exp: split the evac across DVE and the idle ACT
                    # so the chunk's critical path halves
                    nc.vector.tensor_copy(ysb[:, 0:512], ps[:, 0:512])
                    nc.scalar.activation(
                        ysb[:, 512:1024], ps[:, 512:1024],
                        mybir.ActivationFunctionType.Copy)
                else:
                    nc.vector.tensor_copy(ysb[:], ps[:])
                store_q[store_n[0] % 3].dma_start(out_e[ss:ss + 128, :],
                                                  ysb[:])
                store_n[0] += 1

            # ================= emission schedule =================
            # early rope passes: swaps on ACT (k) / DVE (q) — they finish
            # before the first exp; later passes: swaps on DVE.
            # k-p1 is only needed from scores kt4 (~4 exps of slack), so
            # the first q chunk-pair completes as early as possible. The
            # pre-exp swaps are split so the serial DVE rope chain stays
            # short: ACT (idle until the first exp) takes three passes,
            # DVE takes q-p1 in parallel.
            rope_pass(wkb, bk, k_rot, 0, nc.scalar)
            rope_pass(wqb, bq, q_rot, 0, nc.vector)
            rope_pass(wqb, bq, q_rot, 1, nc.vector)
            rope_pass(wkb, bk, k_rot, 1, nc.scalar)

            for kt in range(8):
                sc_kt(0, kt)
            rope_pass(wkb, bk, k_rot, 2, nc.vector)
            rope_pass(wkb, bk, k_rot, 3, nc.vector)
            for kt in range(8, 12):
                sc_kt(0, kt)
            rope_pass(wqb, bq, q_rot, 2, nc.vector)
            for kt in range(12, 16):
                sc_kt(0, kt)
            rope_pass(wqb, bq, q_rot, 3, nc.vector)
            v_tiles(range(NKT))
            p1_cm.__exit__(None, None, None)

            # pair-0 cc0 attn@v rides the pair-0 exp stream
            av_sweep(0, 0)
            # kick off pair-1 scores so ACT never idles at the pair
            # boundary; drain pair-0 cc1 (not exp-gated, fast) after
            sc_kt(1, 0)
            sc_kt(1, 1)
            norm_cc(0, 0)
            av_open(0, 1)
            for kt in range(NKT):
                av_kt(0, 1, kt)
            av_evac(0, 1)
            norm_cc(0, 1)
            # pair-1: cc0's chain pair rides the exp stream; pair-0 proj
            # chunks fill the PE gaps (transiently borrowing the third
            # score PSUM buffer)
            av_open(1, 0)
            for kt in range(2, 16):
                sc_kt(1, kt)
                av_kt(1, 0, kt - 2)
                if kt % 2 == 0:
                    proj_chunk(0, (kt - 2) // 8, ((kt - 2) // 2) % 4)
            proj_chunk(0, 1, 3)
            for kt in range(14, 16):
                av_kt(1, 0, kt)
            av_evac(1, 0)
            norm_cc(1, 0)
            # tail: cc1 chains in one big-pool tile, interleaved with
            # pair-1 cc0 proj chunks; evac work split across DVE and the
            # now-idle ACT engine. (Opening the cc1 chains mid-stream to
            # ride the exp tail was measured: the score-buffer steal
            # stretches the exp stream more than the tail saves.)
            av_open(1, 1, wide=True)
            for kt in range(8):
                av_kt(1, 1, kt)
            proj_chunk(1, 0, 0, tail=True)
            proj_chunk(1, 0, 1, tail=True)
            for kt in range(8, 16):
                av_kt(1, 1, kt)
            proj_chunk(1, 0, 2, tail=True)
            av_evac(1, 1, on_act=True)
            proj_chunk(1, 0, 3, tail=True)
            norm_cc(1, 1)
            for u in range(4):
                proj_chunk(1, 1, u, tail=True)
            ptp_cm.__exit__(None, None, None)

    nc.compile()
    return nc


def make_in_maps(x, sin, cos, W_qkv, b_qkv):
    x = np.asarray(x, np.float32)
    sin = np.asarray(sin, np.float32)
    cos = np.asarray(cos, np.float32)
    W_qkv = np.asarray(W_qkv, np.float32)
    b_qkv = np.asarray(b_qkv, np.float32)

    xT = np.ascontiguousarray(x.T).astype(ml_dtypes.bfloat16)
    # sin/cos halves are duplicated (ang = concat([ang, ang])); rows are
    # [h0 d0:32, h0 d32:64, h1 d0:32, h1 d32:64] -> 4x tile of the
    # first-half columns works for cos. The rotate-half sign pattern is
    # [-s, +s, -s, +s] per 32-row block.
    cosT = np.ascontiguousarray(np.tile(cos[:, :32].T, (4, 1))).astype(
        ml_dtypes.bfloat16)
    sin32 = sin[:, :32].T
    sinTs = np.ascontiguousarray(
        np.concatenate([-sin32, sin32, -sin32, sin32], 0)).astype(
            ml_dtypes.bfloat16)

    scale = 1.0 / np.sqrt(np.float32(D))
    Wq = W_qkv[0:DIM] * scale
    Wk = W_qkv[DIM:2 * DIM]
    Wv = W_qkv[2 * DIM:3 * DIM]
    bq_full = b_qkv[0:DIM] * scale
    bk_full = b_qkv[DIM:2 * DIM]
    bv_full = b_qkv[2 * DIM:3 * DIM]

    def bundle(wT):
        # [1024, 128] lhsT layout -> [128, 8*128] col-block bundle
        return np.ascontiguousarray(
            wT.reshape(NDT, 128, DL).transpose(1, 0, 2).reshape(128, DIM)
        ).astype(ml_dtypes.bfloat16)

    in_maps = []
    for core in range(N_CORES):
        h0, h1 = 2 * core, 2 * core + 1

        def head_rows(W):
            return np.concatenate([W[h0 * D:(h0 + 1) * D],
                                   W[h1 * D:(h1 + 1) * D]], 0)

        wq_c = head_rows(Wq)
        wk_c = head_rows(Wk)
        wv_c = head_rows(Wv)
        bq_c = head_rows(bq_full[:, None])
        bk_c = head_rows(bk_full[:, None])
        bv_row = head_rows(bv_full[:, None])[:, 0]
        bvb_c = np.broadcast_to(bv_row[None, :], (DL, DL))
        in_maps.append({
            "xT": xT,
            "wqB": bundle(np.ascontiguousarray(wq_c.T)),
            "wkB": bundle(np.ascontiguousarray(wk_c.T)),
            "wvB": bundle(np.ascontiguousarray(wv_c.T)),
            "cosT": cosT,
            "sinTs": sinTs,
            "bq": np.ascontiguousarray(bq_c),
            "bk": np.ascontiguousarray(bk_c),
            "bvb": np.ascontiguousarray(bvb_c),
        })
    return in_maps


def add_wp(in_maps, W_proj):
    W_proj = np.asarray(W_proj, np.float32)
    for core in range(N_CORES):
        cols = slice(core * DL, (core + 1) * DL)
        in_maps[core]["wpT"] = np.ascontiguousarray(
            W_proj[:, cols].T).astype(ml_dtypes.bfloat16)
    return in_maps


_NC_CACHE = {}


def kernel(x, sin, cos, W_qkv, b_qkv, W_proj, b_proj):
    if "nc" not in _NC_CACHE:
        _NC_CACHE["nc"] = build()
    nc = _NC_CACHE["nc"]
    in_maps = add_wp(make_in_maps(x, sin, cos, W_qkv, b_qkv), W_proj)
    res = bass_utils.run_bass_kernel_spmd(
        nc, in_maps, core_ids=list(range(N_CORES)))
    y = np.zeros((S, DIM), np.float64)
    for core in range(N_CORES):
        y += res.results[core]["out"].astype(np.float64)
    y += np.asarray(b_proj, np.float32)[None, :].astype(np.float64)
    return y.astype(np.float32)
